# revision 15
# baseline (speedup 1.0000x reference)
"""Trainium2 Bass kernel for nn_HaarDecomposition2D.

The reference computes a 9-level redundant "diagonal Haar" decomposition of a
(8,3,512,512) image batch, emitting per-level full-resolution detail images
plus the final low-pass, concatenated to (8,30,512,512).

Algebraic structure (verified bit-exact vs the reference):
the one-level transform is a projection — its low-pass output is a fixed
point of the level map, so every detail level >= 2 is exactly zero and
low_9 == low_1.  The kernel therefore computes det_1 and low_1 only.
Channels 3..26 are exactly zero; run_bass_kernel_spmd's contract pre-zeros
ExternalOutput buffers on both the native path (out_maps) and the axon/PJRT
path (donated zero buffers), so the kernel does not write them.  kernel()
additionally re-asserts those zeros host-side.

Sharding: pure batch data-parallel, batch item b -> NeuronCore b (8 cores).

Per-core layout: for each channel the 512x512 image is loaded as an SBUF
tile [128 partitions, 2048] where partition I holds image rows 4I..4I+3,
so every 2x2-block pairing becomes a free-dimension strided op.  The four
half-resolution Haar sums are produced interleaved (dd at even, ud at odd
columns) into EI/OI tiles [128, 512]:

  EI[2t]   = X[r0,2t]   + X[r1,2t+1]      (E_dd; rows 4I,4I+1)
  EI[2t+1] = X[r0,2t+1] + X[r1,2t]        (E_ud)
  OI[2t]   = X[r2,2t]   + X[r3,2t+1]      (O_dd; rows 4I+2,4I+3)
  OI[2t+1] = X[r2,2t+1] + X[r3,2t]        (O_ud)
  OI *= 0.25

With that interleaving, each output row-position r in {0..3} is a single
fused op with a fully CONTIGUOUS [128, 512] write:

  low[r]  = 0.25*perm(EI, a_r) + perm(OI, b_r)
  det[r]  = 0.25*perm(EI, a_r) - perm(OI, b_r)
  (a_r, b_r) = (0,2), (1,3), (2,0), (3,1)

where perm(t, m)[j] = t[(j & ~3) | ((j & 3) ^ m)] — XOR permutations of
4-blocks, expressed as (negative-stride) access patterns:
  m=0: identity      m=1: [[2,256],[-1,2]] off+1
  m=2: [[4,128],[-2,2],[1,2]] off+2        m=3: [[4,128],[-1,4]] off+3
"""

import sys

if "/opt/trn_rl_repo" not in sys.path:
    sys.path.insert(0, "/opt/trn_rl_repo")

import numpy as np

_NCORES = 8
_C = 3
_H = 512
_W = 512
_OC = 30  # 9 detail levels * 3 channels + 3 low-pass channels

_nc_cache = {}


def _build_nc():
    """Build the per-core Bass program: in x[3,512,512] -> out[30,512,512]."""
    import concourse.bacc as bacc
    import concourse.bass as bass
    import concourse.mybir as mybir
    from concourse.tile import TileContext

    fp32 = mybir.dt.float32
    A = mybir.AluOpType

    nc = bacc.Bacc("TRN2", target_bir_lowering=False, debug=False,
                   enable_asserts=False)

    xt = nc.dram_tensor("x", [_C, _H, _W], fp32, kind="ExternalInput")
    ot = nc.dram_tensor("out", [_OC, _H, _W], fp32, kind="ExternalOutput")

    def img4(ap):
        # [512,512] image -> [128, 2048]: partition I holds rows 4I..4I+3
        return ap.rearrange("(p q) w -> p (q w)", q=4)

    def xor_view(tile, m):
        # perm(t, m)[j] = t[(j & ~3) | ((j & 3) ^ m)] on a [128, 512] tile.
        # m=2 would need a 4D AP (verifier allows <=3D); callers split it.
        base = tile[:]
        if m == 0:
            return base
        if m == 1:
            ap = [[512, 128], [2, 256], [-1, 2]]
        else:  # m == 3
            ap = [[512, 128], [4, 128], [-1, 4]]
        return bass.AP(tile.tensor, base.offset + m, ap)

    def pair_view(base, off):
        # elements {4J+off, 4J+off+1} of an AP with a contiguous 512-elem
        # free window: 3D pattern, 8-byte runs
        return bass.AP(base.tensor, base.offset + off,
                       [list(base.ap[0]), [4, 128], [1, 2]])

    dma_engines = [nc.sync, nc.scalar]
    dma_i = 0

    def dma(out, in_):
        nonlocal dma_i
        dma_engines[dma_i % 2].dma_start(out=out, in_=in_)
        dma_i += 1

    with TileContext(nc) as tc:
        with tc.tile_pool(name="img", bufs=3) as img_pool, \
             tc.tile_pool(name="outp", bufs=3) as out_pool, \
             tc.tile_pool(name="eo", bufs=3) as eo_pool:

            for c in range(_C):
                X = img_pool.tile([128, 2048], fp32, tag="X")
                dma(X[:], img4(xt[c]))

                EI = eo_pool.tile([128, 512], fp32, tag="EI")
                OI = eo_pool.tile([128, 512], fp32, tag="OI")

                v = nc.vector
                # E_dd / E_ud interleaved into EI; O_dd / O_ud into OI
                v.tensor_tensor(out=EI[:, 0:511:2], in0=X[:, 0:512:2],
                                in1=X[:, 513:1024:2], op=A.add)
                v.tensor_tensor(out=EI[:, 1:512:2], in0=X[:, 1:512:2],
                                in1=X[:, 512:1024:2], op=A.add)
                v.tensor_tensor(out=OI[:, 0:511:2], in0=X[:, 1024:1536:2],
                                in1=X[:, 1537:2048:2], op=A.add)
                v.tensor_tensor(out=OI[:, 1:512:2], in0=X[:, 1025:1536:2],
                                in1=X[:, 1536:2048:2], op=A.add)
                # pre-scale OI so each output row is one fused op:
                # row = (EI_view * 0.25) +/- OI_view
                v.tensor_scalar_mul(OI[:], OI[:], 0.25)

                D = out_pool.tile([128, 2048], fp32, tag="D")
                L = out_pool.tile([128, 2048], fp32, tag="L")
                for r, (a, b) in enumerate([(0, 2), (1, 3), (2, 0), (3, 1)]):
                    for out_t, op1 in ((L, A.add), (D, A.subtract)):
                        dst = out_t[:, r * 512:(r + 1) * 512]
                        if r % 2 == 1:
                            # a,b in {1,3}: single op, 3D XOR views
                            v.scalar_tensor_tensor(
                                out=dst, in0=xor_view(EI, a), scalar=0.25,
                                in1=xor_view(OI, b), op0=A.mult, op1=op1)
                        else:
                            # a,b in {0,2}: XOR2 needs a 4D AP; split into
                            # two pair-granular halves (all views 3D)
                            for h in (0, 2):
                                v.scalar_tensor_tensor(
                                    out=pair_view(dst, h),
                                    in0=pair_view(EI[:], h ^ a), scalar=0.25,
                                    in1=pair_view(OI[:], h ^ b),
                                    op0=A.mult, op1=op1)

                dma(img4(ot[c]), D[:])
                dma(img4(ot[27 + c]), L[:])

    nc.finalize()
    return nc


def _get_nc():
    if "nc" not in _nc_cache:
        _nc_cache["nc"] = _build_nc()
    return _nc_cache["nc"]


def run_spmd(x, **kwargs):
    """Run the SPMD kernel on 8 cores; returns (stacked_output, BassKernelResults)."""
    from concourse.bass_utils import run_bass_kernel_spmd

    x = np.ascontiguousarray(np.asarray(x, dtype=np.float32))
    assert x.shape == (_NCORES, _C, _H, _W), x.shape
    nc = _get_nc()
    in_maps = [{"x": np.ascontiguousarray(x[b])} for b in range(_NCORES)]
    res = run_bass_kernel_spmd(nc, in_maps, core_ids=list(range(_NCORES)),
                               **kwargs)
    out = np.stack([res.results[b]["out"] for b in range(_NCORES)], axis=0)
    # channels 3..26 are mathematically zero; the device relies on the
    # pre-zeroed output contract — re-assert host-side for safety.
    out[:, 3:27] = 0.0
    return out, res


def kernel(x):
    out, _ = run_spmd(x)
    return out
